# revision 57
# baseline (speedup 1.0000x reference)
"""Complex LayerNorm TRN2 kernel (nn_Complex_LayerNorm).

Math (per row r over embed dim D, per feature d):
    whiten:  y = C(r) @ (x - mu(r)),  C = inv(sqrtm(cov2x2))
    recolor: z = Wsqrt(d) @ y + bias(d)

Per-core pipeline (batch sharded 1 core per batch element, 32 row-tiles):
  - inputs downcast to fp16 on host (halves input DMA traffic), xr|xi
    packed into one [rows, 2D] tensor (one input DMA per tile)
  - moments: bn_stats on DVE (xr, xi), cross-cov on DVE via
    scalar_tensor_tensor accumulate
  - stage1 (PE): psum1 = xr_blk^T @ [diag(i00)|diag(i01)]
                       + xi_blk^T @ [diag(i01)|diag(i11)]
    -> whitened-without-mean yT in feature-major layout (fp16 diags)
  - stage2 (PE): psum2 = yrT^T @ W1[b] + yiT^T @ W2[b] + UVB^T @ OX
    where W1/W2 recolor + transpose back + interleave (zr,zi), and the
    rank-3 UVB/OX term adds bias and subtracts the recolored row means
    (UVB rows = [1, -u, -v] built via a small DMA transpose of the stats)
  - input DMAs are issued PF=7 tiles ahead so the SP sequencer's
    in-order queue (input DMA, UVB transpose, output DMA) never starves
    the stats pipeline; all four PSUM->SBUF copies run on Act.
"""

import numpy as np

import concourse.bacc as bacc
import concourse.tile as tile
from concourse import mybir
from concourse import bass_utils

F32 = mybir.dt.float32
F16 = mybir.dt.float16
AL = mybir.AluOpType
AF = mybir.ActivationFunctionType

B, S, D = 8, 4096, 1024
NT = S // 128        # 32 row tiles per core
NB = D // 128        # 8 feature blocks
C1 = 1024.0 / 1023.0  # unbiased variance correction (torch.var ddof=1)


def _build_nc(nt=NT):
    rows = nt * 128
    nc = bacc.Bacc("TRN2")

    xin_d = nc.dram_tensor("x_in", (rows, 2 * D), F16, kind="ExternalInput").ap()
    w1_d = nc.dram_tensor("w1c", (NB, 128, 256), F16, kind="ExternalInput").ap()
    w2_d = nc.dram_tensor("w2c", (NB, 128, 256), F16, kind="ExternalInput").ap()
    ox_d = nc.dram_tensor("oxc", (3, 2, 1024), F16, kind="ExternalInput").ap()
    id_d = nc.dram_tensor("identh", (128, 128), F16, kind="ExternalInput").ap()
    nid_d = nc.dram_tensor("nidenth", (128, 128), F16, kind="ExternalInput").ap()
    out_d = nc.dram_tensor("out", (rows, 2 * D), F16, kind="ExternalOutput").ap()

    with tile.TileContext(nc) as tc:
        with (
            tc.tile_pool(name="const", bufs=1) as pc,
            tc.tile_pool(name="xin", bufs=8) as px,
            tc.tile_pool(name="scratch", bufs=3) as psc,
            tc.tile_pool(name="stats", bufs=6) as pst,
            tc.tile_pool(name="diag", bufs=4) as pdg,
            tc.tile_pool(name="uvb", bufs=4) as puv,
            tc.tile_pool(name="yt", bufs=6) as pyt,
            tc.tile_pool(name="outp", bufs=4) as pout,
            tc.tile_pool(name="ps1", bufs=2, space="PSUM") as ps1,
            tc.tile_pool(name="ps2", bufs=2, space="PSUM") as ps2,
        ):
            # ---- constants ----
            w1c = pc.tile([128, NB, 256], F16)
            nc.sync.dma_start(out=w1c, in_=w1_d.rearrange("b p n -> p b n"))
            w2c = pc.tile([128, NB, 256], F16)
            nc.sync.dma_start(out=w2c, in_=w2_d.rearrange("b p n -> p b n"))
            oxc = pc.tile([3, 2, 1024], F16)
            nc.sync.dma_start(out=oxc, in_=ox_d)
            identh = pc.tile([128, 128], F16)
            nc.sync.dma_start(out=identh, in_=id_d)
            nidenth = pc.tile([128, 128], F16)
            nc.sync.dma_start(out=nidenth, in_=nid_d)

            vts = nc.vector.tensor_scalar

            xts = {}       # tile idx -> xt sbuf tile
            fronts = {}    # tile idx -> (xt, DG, UVB)
            PF = 7         # input DMA issued PF tiles ahead of the front-end
            LEAD = 0       # front-end runs LEAD tiles ahead of the back-end

            def _fetch(j):
                xt = px.tile([128, 2 * D], F16, tag="xt")
                nc.sync.dma_start(out=xt, in_=xin_d[j * 128 : (j + 1) * 128, :])
                xts[j] = xt

            def _front(f):
                if f + PF < nt:
                    _fetch(f + PF)
                xt = xts[f]
                xr = xt[:, 0:D]
                xi = xt[:, D : 2 * D]

                # ---- moments ----
                # ST columns: 0 mu_r, 1 var_r(b), 2 mu_i, 3 var_i(b), 4 sum(xr*xi),
                # 5 m, 6 cov, 7 vru, 8 viu, 9 q2, 10 det, 11 s, 12 2s, 13 vsum_b,
                # 14 t, 15 st, 16 inv, 17 i00, 18 i01p, 19 i11, 20 u1, 21 v1,
                # 22 ones, 23 t_u(=-u), 24 t_v(=+v)
                ST = pst.tile([128, 26], F32, tag="st")
                bsr = pst.tile([128, 2, 6], F32, tag="bsr")
                nc.vector.bn_stats(out=bsr[:, 0, :], in_=xr[:, 0:512])
                nc.vector.bn_stats(out=bsr[:, 1, :], in_=xr[:, 512:1024])
                nc.vector.bn_aggr(out=ST[:, 0:2], in_=bsr)
                bsi = pst.tile([128, 2, 6], F32, tag="bsi")
                nc.vector.bn_stats(out=bsi[:, 0, :], in_=xi[:, 0:512])
                nc.vector.bn_stats(out=bsi[:, 1, :], in_=xi[:, 512:1024])
                nc.vector.bn_aggr(out=ST[:, 2:4], in_=bsi)
                prod = psc.tile([128, D], F16, tag="prod")
                nc.vector.scalar_tensor_tensor(
                    out=prod,
                    in0=xr,
                    scalar=1.0,
                    in1=xi,
                    op0=AL.mult,
                    op1=AL.mult,
                    accum_out=ST[:, 4:5],
                )

                # ---- per-row 2x2 whitening coefficients (GPSIMD, except the
                # two Sqrts on Act and the reciprocal on DVE) ----
                gts = nc.gpsimd.tensor_scalar
                # m = mu_r*mu_i ; cov = sri/D - m
                gts(out=ST[:, 5:6], in0=ST[:, 0:1], scalar1=ST[:, 2:3], scalar2=None, op0=AL.mult)
                gts(out=ST[:, 6:7], in0=ST[:, 4:5], scalar1=1.0 / D, scalar2=ST[:, 5:6], op0=AL.mult, op1=AL.subtract)
                # vru/viu = unbiased variances
                gts(out=ST[:, 7:8], in0=ST[:, 1:2], scalar1=C1, scalar2=None, op0=AL.mult)
                gts(out=ST[:, 8:9], in0=ST[:, 3:4], scalar1=C1, scalar2=None, op0=AL.mult)
                # q2 = cov^2 ; det = vru*viu - q2
                gts(out=ST[:, 9:10], in0=ST[:, 6:7], scalar1=ST[:, 6:7], scalar2=None, op0=AL.mult)
                gts(out=ST[:, 10:11], in0=ST[:, 7:8], scalar1=ST[:, 8:9], scalar2=ST[:, 9:10], op0=AL.mult, op1=AL.subtract)
                # s = sqrt(det); 2s = sqrt(4*det) (second Act op, avoiding a
                # Pool round-trip between the two sqrts); vsum_b;
                # t = sqrt(C1*vsum_b + 2s)
                nc.scalar.activation(out=ST[:, 11:12], in_=ST[:, 10:11], func=AF.Sqrt)
                nc.scalar.activation(out=ST[:, 12:13], in_=ST[:, 10:11], func=AF.Sqrt, scale=4.0)
                gts(out=ST[:, 13:14], in0=ST[:, 1:2], scalar1=ST[:, 3:4], scalar2=None, op0=AL.add)
                nc.scalar.activation(out=ST[:, 14:15], in_=ST[:, 13:14], func=AF.Sqrt, bias=ST[:, 12:13], scale=C1)
                # inv = 1/(t*s)
                gts(out=ST[:, 15:16], in0=ST[:, 14:15], scalar1=ST[:, 11:12], scalar2=None, op0=AL.mult)
                nc.vector.reciprocal(out=ST[:, 16:17], in_=ST[:, 15:16])
                # i00 = (viu+s)*inv ; i01p = cov*inv (= -i01) ; i11 = (vru+s)*inv
                gts(out=ST[:, 17:18], in0=ST[:, 8:9], scalar1=ST[:, 11:12], scalar2=ST[:, 16:17], op0=AL.add, op1=AL.mult)
                gts(out=ST[:, 18:19], in0=ST[:, 6:7], scalar1=ST[:, 16:17], scalar2=None, op0=AL.mult)
                gts(out=ST[:, 19:20], in0=ST[:, 7:8], scalar1=ST[:, 11:12], scalar2=ST[:, 16:17], op0=AL.add, op1=AL.mult)
                # ---- per-row diagonal matrices (fp16, on GPSIMD) ----
                # Emitted BEFORE the t_u/t_v ops: DG only needs i00/i01p/i11
                # and gates stage1, so it must not sit behind the UVB tail of
                # the chain in Pool's in-order stream.
                DG = pdg.tile([128, 3, 128], F16, tag="dg")
                nc.gpsimd.tensor_scalar(out=DG[:, 0, :], in0=identh, scalar1=ST[:, 17:18], scalar2=None, op0=AL.mult)
                nc.gpsimd.tensor_scalar(out=DG[:, 1, :], in0=nidenth, scalar1=ST[:, 18:19], scalar2=None, op0=AL.mult)
                nc.gpsimd.tensor_scalar(out=DG[:, 2, :], in0=identh, scalar1=ST[:, 19:20], scalar2=None, op0=AL.mult)

                # t_u = -u = mu_i*i01p - mu_r*i00 ; t_v = +v = mu_i*i11 - mu_r*i01p
                gts(out=ST[:, 20:21], in0=ST[:, 0:1], scalar1=ST[:, 17:18], scalar2=None, op0=AL.mult)
                gts(out=ST[:, 23:24], in0=ST[:, 2:3], scalar1=ST[:, 18:19], scalar2=ST[:, 20:21], op0=AL.mult, op1=AL.subtract)
                gts(out=ST[:, 21:22], in0=ST[:, 0:1], scalar1=ST[:, 18:19], scalar2=None, op0=AL.mult)
                gts(out=ST[:, 24:25], in0=ST[:, 2:3], scalar1=ST[:, 19:20], scalar2=ST[:, 21:22], op0=AL.mult, op1=AL.subtract)
                nc.vector.memset(ST[:, 22:23], 1.0)

                # ---- UVB = DMA-xbar transpose of [ones, t_u, t_v] ----
                # STH cols 3:128 are never written; the transposed garbage
                # lands in UVB rows 3:128 which are never read.
                STH = pst.tile([128, 128], F16, tag="sth")
                vts(out=STH[:, 0:3], in0=ST[:, 22:25], scalar1=1.0, scalar2=None, op0=AL.mult)
                UVB = puv.tile([128, 128], F16, tag="uvb")
                nc.sync.dma_start_transpose(out=UVB, in_=STH)
                fronts[f] = (xt, DG, UVB)

            def _back(i):
                r0 = i * 128
                xt, DG, UVB = fronts.pop(i)
                del xts[i]
                xr = xt[:, 0:D]
                xi = xt[:, D : 2 * D]

                # ---- stage 1: whiten + transpose (half-tiles of 4 blocks) ----
                yts = []
                for g in range(2):
                    p1 = ps1.tile([128, 1024], F32, tag="p1")
                    for j in range(4):
                        b = 4 * g + j
                        o = p1[:, 256 * j : 256 * (j + 1)]
                        xr_blk = xr[:, 128 * b : 128 * (b + 1)]
                        xi_blk = xi[:, 128 * b : 128 * (b + 1)]
                        nc.tensor.matmul(o, xr_blk, DG[:, 0:2, :], start=True, stop=False)
                        nc.tensor.matmul(o, xi_blk, DG[:, 1:3, :], start=False, stop=True)
                    yt = pyt.tile([128, 1024], F16, tag="yt")
                    if g == 0:
                        # split: stage2-h0's first matmuls only need cols
                        # 0:512, so let them start after the first half-copy
                        nc.scalar.copy(out=yt[:, 0:512], in_=p1[:, 0:512])
                        nc.scalar.copy(out=yt[:, 512:1024], in_=p1[:, 512:1024])
                    else:
                        nc.scalar.copy(out=yt, in_=p1)
                    yts.append(yt)

                # ---- stage 2: recolor + transpose back + offsets/bias ----
                out_sb = pout.tile([128, 2 * D], F16, tag="osb")
                for h in range(2):
                    p2 = ps2.tile([128, 1024], F32, tag="p2")
                    for k in range(2):
                        for j2 in range(2):
                            j = 2 * k + j2
                            b = 4 * h + j
                            yt = yts[b // 4]
                            c0 = 256 * (b % 4)
                            yrT = yt[:, c0 : c0 + 128]
                            yiT = yt[:, c0 + 128 : c0 + 256]
                            o = p2[:, 256 * j : 256 * (j + 1)]
                            nc.tensor.matmul(o, yrT, w1c[:, b, :], start=(j2 == 0), stop=False)
                            nc.tensor.matmul(o, yiT, w2c[:, b, :], start=False, stop=False)
                        nc.tensor.matmul(
                            p2[:, 512 * k : 512 * (k + 1)],
                            UVB[0:3, :],
                            oxc[:, h, 512 * k : 512 * (k + 1)],
                            start=False,
                            stop=True,
                        )
                    nc.scalar.copy(out=out_sb[:, 1024 * h : 1024 * (h + 1)], in_=p2)
                # Output DMA on the Act HWDGE queue: its dependency (out_sb)
                # is written by Act itself, so it never blocks the queue --
                # and the SP queue (inputs + UVB transposes) never stalls
                # behind the output's long semaphore wait.
                nc.scalar.dma_start(out=out_d[r0 : r0 + 128, :], in_=out_sb)

            for j in range(min(PF, nt)):
                _fetch(j)
            for f in range(min(LEAD, nt)):
                _front(f)
            for i in range(nt):
                if i + LEAD < nt:
                    _front(i + LEAD)
                _back(i)

    nc.finalize()
    return nc


_NC = None


def _get_nc():
    global _NC
    if _NC is None:
        _NC = _build_nc()
    return _NC


def _host_consts(weights, bias_real, bias_imag):
    w = weights.astype(np.float64)
    wr = w[:, 0, 0] ** 2
    wi = w[:, 1, 0] ** 2
    sig = 1.0 / (1.0 + np.exp(-w[:, 2, 0]))
    wc = (sig - 0.5) * 2.0 * np.sqrt(wr * wi)
    sw = np.sqrt(wr * wi - wc * wc)
    tw = np.sqrt(wr + wi + 2.0 * sw)
    w00 = (wr + sw) / tw
    w01 = wc / tw
    w11 = (wi + sw) / tw

    jj = np.arange(128)
    W1 = np.zeros((NB, 128, 256), np.float16)
    W2 = np.zeros((NB, 128, 256), np.float16)
    for b in range(NB):
        f = 128 * b + jj
        W1[b, jj, 2 * jj] = w00[f]
        W1[b, jj, 2 * jj + 1] = w01[f]
        W2[b, jj, 2 * jj] = w01[f]
        W2[b, jj, 2 * jj + 1] = w11[f]

    # OX rows: contribution = 1*row0 + t_u*row1 + t_v*row2 with t_u=-u, t_v=+v
    # must equal br - u*w00 - v*w01 (even cols) / bi - u*w01 - v*w11 (odd).
    OX = np.zeros((3, 2, 1024), np.float16)
    for h in range(2):
        f = 512 * h + np.arange(512)
        OX[0, h, 0::2] = bias_real[f]
        OX[0, h, 1::2] = bias_imag[f]
        OX[1, h, 0::2] = w00[f]
        OX[1, h, 1::2] = w01[f]
        OX[2, h, 0::2] = -w01[f]
        OX[2, h, 1::2] = -w11[f]

    I = np.eye(128, dtype=np.float16)
    return {
        "w1c": W1,
        "w2c": W2,
        "oxc": OX,
        "identh": I,
        "nidenth": -I,
    }


def _run(x_real, x_imag, weights, bias_real, bias_imag, trace=False):
    nc = _get_nc()
    consts = _host_consts(
        np.asarray(weights, np.float64),
        np.asarray(bias_real, np.float64),
        np.asarray(bias_imag, np.float64),
    )
    xin = np.empty((B, S, 2 * D), np.float16)
    xin[:, :, :D] = np.asarray(x_real)
    xin[:, :, D:] = np.asarray(x_imag)
    in_maps = [{"x_in": xin[c], **consts} for c in range(B)]
    res = bass_utils.run_bass_kernel_spmd(
        nc, in_maps, core_ids=list(range(B)), trace=trace
    )
    out = np.empty((B, S, D), np.complex64)
    for c in range(B):
        zz = np.asarray(res.results[c]["out"]).astype(np.float32)
        out[c] = np.ascontiguousarray(zz).view(np.complex64)
    return out, res


def kernel(x_real, x_imag, weights, bias_real, bias_imag):
    out, _ = _run(x_real, x_imag, weights, bias_real, bias_imag, trace=False)
    return out
